# revision 1
# baseline (speedup 1.0000x reference)
"""BiDAF2 attention kernel for Trainium2, 8-core data parallel over batch.

reference (per batch b):
  w1h[s,l] = h[s,:] @ w1_w[l,:] + w1_b[l]
  w2q[t,l] = q[t,:] @ w2_w[l,:] + w2_b[l]
  a[s,t]   = w1h[s,t] + w2q[t,s] + h[s,:]@q[t,:]
  p        = softmax_t(a);  c[s,:] = p[s,:] @ q
  m[s]     = max_t a[s,t];  p2 = softmax_s(m)
  out      = concat([h, c, h*c, (h*p2)*c], axis=-1)

Strategy per core (2 batches):
  - All big matmuls in fp16 on the PE (fp32 PSUM accumulation). The dominant
    h@q^T logit term optionally uses a 3-pass hi/lo fp16 split (SPLIT3) for
    near-fp32 logit accuracy.
  - a lives only in PSUM: w1_b folded in via a K=1 matmul; row max via DVE
    reduce_max(negate=True) straight off PSUM; exp on ACT straight off PSUM
    with fused row-sum accumulation; p written directly as fp16.
  - p transposed per s-tile with one xbar transpose-DMA; c matmuls accumulate
    over the 8 t-chunks in PSUM; softmax normalization folded into the c
    epilogue scale (1/Z per row).
  - w2_b does not change softmax_t(a); the row max is corrected by +w2_b
    afterwards, before the p2 softmax.
  - p2 (softmax over the 1024 row maxes, a cross-partition reduction) via a
    4KB DRAM-scratch rearrange to a single-partition row, softmaxed there,
    scattered back to per-partition scalars.
  - out4 = (h*c)*p2 runs on ACT (activation Copy with a per-partition scale
    AP) so the deferred p2 dependency never blocks the DVE FIFO.
"""

import os
import sys

for _p in ("/opt/trn_rl_repo", "/root/.axon_site/_ro/trn_rl_repo"):
    if os.path.isdir(_p) and _p not in sys.path:
        sys.path.append(_p)

from contextlib import ExitStack

import numpy as np

import concourse.bass as bass
import concourse.tile as tile
from concourse import bacc, mybir
from concourse.bass_utils import run_bass_kernel_spmd

B, L, D = 16, 1024, 768
NCORES = 8
BL = B // NCORES  # batches per core
P = 128
KD = D // P  # 6 d-chunks
NT = L // P  # 8 t-chunks == 8 s-tiles
F16 = mybir.dt.float16
F32 = mybir.dt.float32
EXP = mybir.ActivationFunctionType.Exp
COPY = mybir.ActivationFunctionType.Copy
AX = mybir.AxisListType.X

SPLIT3 = True  # 3-pass hi/lo fp16 split for the h@q^T logit term
REPEAT = 1  # benchmarking aid: run the whole body REPEAT times via For_i


def _emit(ctx: ExitStack, tc: tile.TileContext, h, q, w1w, w1b, w2w, w2b, out):
    if REPEAT > 1:
        with tc.For_i(0, REPEAT, 1):
            _emit_once(ctx, tc, h, q, w1w, w1b, w2w, w2b, out)
    else:
        _emit_once(ctx, tc, h, q, w1w, w1b, w2w, w2b, out)


def _emit_once(ctx: ExitStack, tc: tile.TileContext, h, q, w1w, w1b, w2w, w2b, out):
    nc = tc.nc
    halves = [(0, 512), (512, 1024)]

    singles = ctx.enter_context(tc.tile_pool(name="singles", bufs=1))
    wT_pool = ctx.enter_context(tc.tile_pool(name="wT", bufs=1))
    nat16 = ctx.enter_context(tc.tile_pool(name="nat16", bufs=2))
    qT_pool = ctx.enter_context(tc.tile_pool(name="qT", bufs=1))
    qlo_pool = ctx.enter_context(tc.tile_pool(name="qlo", bufs=1))
    qnat_pool = ctx.enter_context(tc.tile_pool(name="qnat", bufs=2))
    h_pool = ctx.enter_context(tc.tile_pool(name="h_all", bufs=1))
    hprep = ctx.enter_context(tc.tile_pool(name="hprep", bufs=2))
    pstream = ctx.enter_context(tc.tile_pool(name="pstream", bufs=2))
    pT_pool = ctx.enter_context(tc.tile_pool(name="pT", bufs=1))
    epil = ctx.enter_context(tc.tile_pool(name="epil", bufs=2))
    smalls = ctx.enter_context(tc.tile_pool(name="smalls", bufs=1))
    dram = ctx.enter_context(tc.tile_pool(name="dram", bufs=2, space="DRAM"))
    psA = ctx.enter_context(tc.tile_pool(name="psA", bufs=2, space="PSUM"))
    psC = ctx.enter_context(tc.tile_pool(name="psC", bufs=2, space="PSUM"))

    # ---- constants ----
    ones1 = singles.tile([1, P], F16)
    nc.vector.memset(ones1, 1.0)
    w1b16 = singles.tile([1, L], F16)
    nc.gpsimd.dma_start(out=w1b16, in_=w1b[None, :])
    w2b_col = singles.tile([P, NT], F32)
    nc.sync.dma_start(out=w2b_col, in_=w2b.rearrange("(c p) -> p c", p=P))

    # w2T: [d_part, d_chunk, t] fp16, via chunked load + cast + xbar transpose.
    # (w1_w is folded into u = q + w1_w per batch, so no w1T.)
    w2T = wT_pool.tile([P, KD, L], F16, tag="w2T")
    for tcn in range(NT):
        w2c = qnat_pool.tile([P, D], F32, tag="qnat")
        nc.sync.dma_start(out=w2c, in_=w2w[tcn * P:(tcn + 1) * P, :])
        w2c16 = qlo_pool.tile([P, D], F16, tag="u16hi")
        nc.gpsimd.tensor_copy(out=w2c16, in_=w2c)
        nc.sync.dma_start(
            out=w2T[:, :, tcn * P:(tcn + 1) * P], in_=w2c16, transpose=True
        )

    for b in range(BL):
        # ---- batch-level q/u prep: u = q + w1_w (fp32), split hi/lo fp16 ----
        q16 = nat16.tile([P, NT, D], F16, tag="nat16")
        qT = qT_pool.tile([P, KD, L], F16, tag="qT")
        uThi = qT_pool.tile([P, KD, L], F16, tag="uThi")
        if SPLIT3:
            uTlo = qT_pool.tile([P, KD, L], F16, tag="uTlo")
        else:
            uTlo = None
        for tcn in range(NT):
            rows = slice(tcn * P, (tcn + 1) * P)
            qnat = qnat_pool.tile([P, D], F32, tag="qnat")
            nc.sync.dma_start(out=qnat, in_=q[b, rows, :])
            u32 = qnat_pool.tile([P, D], F32, tag="u32")
            nc.sync.dma_start(out=u32, in_=w1w[rows, :])
            nc.vector.tensor_add(u32, u32, qnat)
            nc.scalar.copy(out=q16[:, tcn, :], in_=qnat)
            u16hi = qlo_pool.tile([P, D], F16, tag="u16hi")
            nc.gpsimd.tensor_copy(out=u16hi, in_=u32)
            nc.sync.dma_start(
                out=qT[:, :, tcn * P:(tcn + 1) * P], in_=q16[:, tcn, :],
                transpose=True,
            )
            nc.sync.dma_start(
                out=uThi[:, :, tcn * P:(tcn + 1) * P], in_=u16hi, transpose=True
            )
            if SPLIT3:
                u16lo = qlo_pool.tile([P, D], F16, tag="u16lo")
                nc.vector.tensor_sub(u16lo, u32, u16hi)
                nc.sync.dma_start(
                    out=uTlo[:, :, tcn * P:(tcn + 1) * P], in_=u16lo,
                    transpose=True,
                )

        h_all = h_pool.tile([P, NT, D], F32)
        m_negcol = smalls.tile([P, NT], F32, tag="m_negcol")
        z_col = smalls.tile([P, NT], F32, tag="z_col")
        r_col = smalls.tile([P, NT], F32, tag="r_col")
        pT_all = pT_pool.tile([P, NT, L], F16)

        # ---- phase A: logits + softmax_t per s-tile ----
        for i in range(NT):
            s0 = i * P
            nc.sync.dma_start(out=h_all[:, i, :], in_=h[b, s0:s0 + P, :])
            h16 = hprep.tile([P, D], F16, tag="h16")
            nc.gpsimd.tensor_copy(out=h16, in_=h_all[:, i, :])
            hT = hprep.tile([P, KD, P], F16, tag="hT")
            nc.sync.dma_start(out=hT, in_=h16, transpose=True)
            if SPLIT3:
                h16lo = hprep.tile([P, D], F16, tag="h16lo")
                nc.vector.tensor_sub(h16lo, h_all[:, i, :], h16)
                hTlo = hprep.tile([P, KD, P], F16, tag="hTlo")
                nc.sync.dma_start(out=hTlo, in_=h16lo, transpose=True)

            ps_a = psA.tile([P, L], F32)
            for t0, t1 in halves:
                nc.tensor.matmul(
                    ps_a[:, t0:t1], ones1, w1b16[:, t0:t1], start=True, stop=False
                )
            for k in range(KD):
                lh = hT[:, k, :]
                for t0, t1 in halves:
                    nc.tensor.matmul(ps_a[:, t0:t1], lh, uThi[:, k, t0:t1],
                                     start=False, stop=False)
                if SPLIT3:
                    for t0, t1 in halves:
                        nc.tensor.matmul(ps_a[:, t0:t1], lh, uTlo[:, k, t0:t1],
                                         start=False, stop=False)
                    llo = hTlo[:, k, :]
                    for t0, t1 in halves:
                        nc.tensor.matmul(ps_a[:, t0:t1], llo, uThi[:, k, t0:t1],
                                         start=False, stop=False)
                lw = w2T[:, k, s0:s0 + P]
                for t0, t1 in halves:
                    nc.tensor.matmul(ps_a[:, t0:t1], lw, qT[:, k, t0:t1],
                                     start=False, stop=(k == KD - 1))

            negm = m_negcol[:, i:i + 1]
            nc.vector.reduce_max(negm, ps_a, axis=AX, negate=True)
            p16 = pstream.tile([P, L], F16, tag="p16")
            nc.scalar.activation(out=p16, in_=ps_a, func=EXP, bias=negm,
                                 scale=1.0, accum_out=z_col[:, i:i + 1])
            nc.sync.dma_start(out=pT_all[:, :, s0:s0 + P], in_=p16,
                              transpose=True)

        # ---- p2 = softmax over all 1024 row maxes (depends on phase A only) ----
        m_true = smalls.tile([P, NT], F32, tag="m_true")
        nc.vector.tensor_sub(m_true, w2b_col, m_negcol)
        m_dram = dram.tile([L], F32, tag="m_dram")
        nc.sync.dma_start(out=m_dram.rearrange("(c p) -> p c", p=P), in_=m_true)
        m_row = smalls.tile([1, L], F32, tag="row_a")
        nc.sync.dma_start(out=m_row, in_=m_dram[None, :])
        negmm = smalls.tile([1, 1], F32, tag="negmm")
        nc.vector.reduce_max(negmm, m_row, axis=AX, negate=True)
        z2 = smalls.tile([1, 1], F32, tag="z2")
        e2 = smalls.tile([1, L], F32, tag="e2")
        nc.scalar.activation(out=e2, in_=m_row, func=EXP, bias=negmm,
                             scale=1.0, accum_out=z2)
        r2 = smalls.tile([1, 1], F32, tag="r2")
        nc.vector.reciprocal(r2, z2)
        p2_row = smalls.tile([1, L], F32, tag="row_a")
        nc.vector.tensor_scalar_mul(p2_row, in0=e2, scalar1=r2)
        p2_dram = dram.tile([L], F32, tag="p2_dram")
        nc.sync.dma_start(out=p2_dram[None, :], in_=p2_row)
        p2_col = smalls.tile([P, NT], F32, tag="p2_col")
        nc.sync.dma_start(out=p2_col, in_=p2_dram.rearrange("(c p) -> p c", p=P))

        # ---- phase B: c = p@q, epilogue ----
        for i in range(NT):
            s0 = i * P
            ps_c = psC.tile([P, D], F32)
            for tcn in range(NT):
                lp = pT_all[:, tcn, s0:s0 + P]
                nc.tensor.matmul(ps_c[:, 0:512], lp, q16[:, tcn, 0:512],
                                 start=(tcn == 0), stop=(tcn == NT - 1))
                nc.tensor.matmul(ps_c[:, 512:D], lp, q16[:, tcn, 512:D],
                                 start=(tcn == 0), stop=(tcn == NT - 1))
            r_i = r_col[:, i:i + 1]
            nc.vector.reciprocal(r_i, z_col[:, i:i + 1])
            # assemble all 4 output sections contiguously -> one 12KB-row DMA
            osec = epil.tile([P, 4, D], F32, tag="osec")
            nc.vector.tensor_copy(osec[:, 0, :], h_all[:, i, :])
            nc.vector.tensor_scalar_mul(osec[:, 1, :], in0=ps_c, scalar1=r_i)
            nc.vector.tensor_mul(osec[:, 2, :], h_all[:, i, :], osec[:, 1, :])
            nc.scalar.activation(out=osec[:, 3, :], in_=osec[:, 2, :], func=COPY,
                                 scale=p2_col[:, i:i + 1])
            nc.sync.dma_start(out=out[b, s0:s0 + P, :], in_=osec)


def build():
    nc = bacc.Bacc()
    h = nc.dram_tensor("h", [BL, L, D], F32, kind="ExternalInput")
    q = nc.dram_tensor("q", [BL, L, D], F32, kind="ExternalInput")
    w1w = nc.dram_tensor("w1_w", [L, D], F32, kind="ExternalInput")
    w1b = nc.dram_tensor("w1_b", [L], F32, kind="ExternalInput")
    w2w = nc.dram_tensor("w2_w", [L, D], F32, kind="ExternalInput")
    w2b = nc.dram_tensor("w2_b", [L], F32, kind="ExternalInput")
    out = nc.dram_tensor("out", [BL, L, 4 * D], F32, kind="ExternalOutput")
    with tile.TileContext(nc) as tc, ExitStack() as ctx:
        _emit(ctx, tc, h[:], q[:], w1w[:], w1b[:], w2w[:], w2b[:], out[:])
    nc.compile()
    return nc


def _in_maps(inputs):
    arr = {k: np.ascontiguousarray(np.asarray(v, np.float32))
           for k, v in inputs.items()}
    maps = []
    for c in range(NCORES):
        sl = slice(c * BL, (c + 1) * BL)
        maps.append({
            "h": arr["h"][sl], "q": arr["q"][sl],
            "w1_w": arr["w1_w"], "w1_b": arr["w1_b"],
            "w2_w": arr["w2_w"], "w2_b": arr["w2_b"],
        })
    return maps


def kernel(**inputs):
    nc = build()
    res = run_bass_kernel_spmd(nc, _in_maps(inputs), core_ids=list(range(NCORES)))
    return np.concatenate([r["out"] for r in res.results], axis=0)


def run_profiled(inputs, **kwargs):
    nc = build()
    res = run_bass_kernel_spmd(
        nc, _in_maps(inputs), core_ids=list(range(NCORES)), trace=True, **kwargs
    )
    out = np.concatenate([r["out"] for r in res.results], axis=0)
    return out, res

